# revision 1
# baseline (speedup 1.0000x reference)
"""Trainium2 Bass kernel for the 16-head masked-attention module.

Math per head (reference):
    q = Q @ Wq.T + bq ; k = K @ Wk.T + bk ; v = V @ Wv.T + bv      [S, 64]
    qk = tril(q @ k.T)                 (zeroed, not -inf)
    scores = log_softmax(qk / 8, axis=0)   (softmax over the QUERY axis,
                                            per key column)
    attn = scores @ v
    out = concat(heads) @ WO.T + bO

Device decomposition (8 cores, 2 heads/core, tensor-parallel over heads,
WO row-sharded; host sums the 8 partial outputs and adds bO):

    scores[s,t] = msc[t,s]/8 - lse[t]   where msc = masked raw qk (transposed
    layout, [t,s]), lse[t] = log(t + sum_{s>=t} exp(msc[t,s]/8))  (the t
    masked zeros contribute exp(0)=1 each).

    attn[s,:] = (1/8)*sum_t msc[t,s] v[t,:]  -  sum_t lse[t] v[t,:]
              = (1/8)*(prefix part + diagonal part) - corr
    with the fully-kept 128-blocks collapsed via rank-64 prefix sums:
        C_m = k_m^T v_m  [64,64];  P_m = sum_{m'<m} C_m'
        prefix part for s-chunk m = (q_m @ P_m)
    Only the 16 diagonal 128x128 triangles are materialized in SBUF.
"""

import numpy as np

S = 2048
D = 1024
NCORES = 8

_CACHE = {}


def _split_multi_waits(nc, mybir, max_waits=1):
    """This walrus build only encodes one sync-wait per instruction; Tile's
    tail drain carries several. Hoist extras onto preceding NoOps."""
    n = 0
    for fn in nc.m.functions:
        for blk in fn.blocks:
            out = []
            changed = False
            for ins in blk.instructions:
                si = getattr(ins, "sync_info", None)
                waits = list(si.on_wait) if (si is not None and si.on_wait) else []
                if len(waits) > max_waits:
                    for w in waits[:-max_waits]:
                        nop = mybir.InstNoOp(
                            name=nc.get_next_instruction_name(), ins=[], outs=[]
                        )
                        nop.engine = ins.engine
                        nop.sync_info = mybir.SyncInfo(on_wait=[w], on_update=[])
                        out.append(nop)
                        n += 1
                    si.on_wait = waits[-max_waits:]
                    changed = True
                out.append(ins)
            if changed:
                blk.instructions = out
    return n


def _build(loop_n=1):
    import concourse.bass as bass
    import concourse.mybir as mybir
    import concourse.tile as tile
    from concourse.bass import ts
    from concourse.masks import make_identity

    F32 = mybir.dt.float32
    BF16 = mybir.dt.bfloat16
    AF = mybir.ActivationFunctionType
    OP = mybir.AluOpType

    nc = bass.Bass("TRN2", num_devices=NCORES, debug=False)

    qt_d = nc.dram_tensor("qt", [D, S], BF16, kind="ExternalInput")
    kt_d = nc.dram_tensor("kt", [D, S], BF16, kind="ExternalInput")
    vt_d = nc.dram_tensor("vt", [D, S], BF16, kind="ExternalInput")
    # packed constants: wbf1 = [wq | wk] (qk-side weights), wbf2 = [wv | wo],
    # cf = [bq | bk | bv | tm | ct] (f32)
    wbf1_d = nc.dram_tensor("wbf1", [128, 2048], BF16, kind="ExternalInput")
    wbf2_d = nc.dram_tensor("wbf2", [128, 2048], BF16, kind="ExternalInput")
    cf_d = nc.dram_tensor("cf", [128, 147], F32, kind="ExternalInput")
    out_d = nc.dram_tensor("out", [S, D], BF16, kind="ExternalOutput")

    with tile.TileContext(nc) as tc:
        with (
            tc.tile_pool(name="singles", bufs=1) as sg,
            tc.tile_pool(name="instream", bufs=6) as instream,
            tc.tile_pool(name="scratch", bufs=2) as scratch,
            tc.tile_pool(name="outs", bufs=8) as outs,
        ):
            # ---- constants (two packed DMAs + one small f32 pack) ----
            wbf1 = sg.tile([128, 2048], BF16, tag="wbf1")
            wbf2 = sg.tile([128, 2048], BF16, tag="wbf2")
            cf = sg.tile([128, 147], F32, tag="cf")
            ident = sg.tile([128, 128], BF16, tag="ident")
            nc.sync.dma_start(wbf1[:], wbf1_d[:])
            nc.sync.dma_start(cf[:], cf_d[:])
            wq_sb = wbf1[:, 0:1024].rearrange("p (o f) -> p o f", f=128)
            wk_sb = wbf1[:, 1024:2048].rearrange("p (o f) -> p o f", f=128)
            wv_sb = wbf2[:, 0:1024].rearrange("p (o f) -> p o f", f=128)
            wo_sb = wbf2[:, 1024:2048]
            bq_sb = cf[:, 0:1]
            bk_sb = cf[:, 1:2]
            bv_sb = cf[:, 2:3]
            tm_sb = cf[:, 3:131]
            ct_sb = cf[:, 131:147]
            make_identity(nc, ident[:])

            # ---- persistent activations ----
            qT = sg.tile([128, S], BF16, tag="qT")   # [dk(2 heads), s]
            kT = sg.tile([128, S], BF16, tag="kT")
            vT = sg.tile([128, S], BF16, tag="vT")
            k_sb = sg.tile([128, 16, 128], BF16, tag="k_sb")  # [t, chunk, dk]
            v_sb = sg.tile([128, 16, 128], BF16, tag="v_sb")  # [t, chunk, dv]
            p_f32 = sg.tile([128, 16, 64], F32, tag="p_f32")  # prefix sums
            p_bf = sg.tile([128, 16, 64], BF16, tag="p_bf")
            zT = sg.tile([128, S], BF16, tag="zT")
            sums_tri = [sg.tile([128, 16], F32, tag=f"s_tri{h}", name=f"s_tri{h}")
                        for h in range(2)]
            sums_sfx = [sg.tile([128, 16], F32, tag=f"s_sfx{h}", name=f"s_sfx{h}")
                        for h in range(2)]
            lse_f = [sg.tile([128, 16], F32, tag=f"lse_f{h}", name=f"lse_f{h}")
                     for h in range(2)]
            lse_b = [sg.tile([128, 16], BF16, tag=f"lse_b{h}", name=f"lse_b{h}")
                     for h in range(2)]
            corr_sb = sg.tile([128, 1], F32, tag="corr")
            tri = [
                [sg.tile([128, 128], BF16, tag=f"tri{h}_{i}", name=f"tri{h}_{i}")
                 for i in range(16)]
                for h in range(2)
            ]
            def emit_body():
                _emit_phases(
                    nc, tc, tile, mybir, ts, F32, BF16, AF, OP,
                    qt_d, kt_d, vt_d, out_d,
                    wq_sb, wk_sb, wv_sb, wo_sb, bq_sb, bk_sb, bv_sb,
                    tm_sb, ct_sb, ident, instream, scratch, outs,
                    qT, kT, vT, k_sb, v_sb, p_f32, p_bf, zT,
                    sums_tri, sums_sfx, lse_f, lse_b, corr_sb, tri,
                    wbf2, wbf2_d,
                )

            if loop_n == 1:
                emit_body()
            else:
                with tc.For_i(0, loop_n, 1):
                    emit_body()

    _split_multi_waits(nc, mybir)
    return nc


def _emit_phases(
    nc, tc, tile, mybir, ts, F32, BF16, AF, OP,
    qt_d, kt_d, vt_d, out_d,
    wq_sb, wk_sb, wv_sb, wo_sb, bq_sb, bk_sb, bv_sb,
    tm_sb, ct_sb, ident, instream, scratch, outs,
    qT, kT, vT, k_sb, v_sb, p_f32, p_bf, zT,
    sums_tri, sums_sfx, lse_f, lse_b, corr_sb, tri,
    wbf2, wbf2_d,
):
    # sums_tri: per-row partial exp-sums from psum tile 1; sums_sfx: tile 2.
    for h in range(2):
        nc.vector.memset(sums_sfx[h][:], 0.0)

    # ---- Phase 1a: K and Q projections (order matters: phase 2 needs both
    # before V arrives, so V's load/compute overlaps the exp grind) ----
    with tc.tile_pool(name="pp", bufs=4, space="PSUM") as pp:
        for name, src_d, w_sb, b_sb, dstT in (
            ("k", kt_d, wk_sb, bk_sb, kT),
            ("q", qt_d, wq_sb, bq_sb, qT),
        ):
            ps = [pp.tile([128, 512], F32, tag="pp", name=f"pp_{name}{j}")
                  for j in range(4)]
            for o in range(8):
                chunk = instream.tile([128, S], BF16, tag="in")
                nc.sync.dma_start(chunk[:], src_d[ts(o, 128), :])
                for j in range(4):
                    nc.tensor.matmul(
                        ps[j][:], w_sb[:, o, :], chunk[:, ts(j, 512)],
                        start=(o == 0), stop=(o == 7),
                    )
            for j in range(4):
                nc.scalar.activation(
                    dstT[:, ts(j, 512)], ps[j][:], AF.Identity,
                    bias=b_sb[:], scale=1.0,
                )

        # ---- Phase 2: score rows [t,s], in-place diag masking, exp sums ----
        with tc.tile_pool(name="pqk", bufs=2, space="PSUM") as pqk:
            # long/short interleave: a short row's qk/mask chain hides under
            # the previous long row's exp sweep
            row_order = []
            for a, b in zip(range(8), range(15, 7, -1)):
                row_order += [a, b]
            for i in row_order:
                j0, r = i // 4, i % 4
                width = (4 - j0) * 512
                w1 = min(width, 1024)
                for h in range(2):
                    hp = slice(64 * h, 64 * h + 64)
                    pq1 = pqk.tile([128, 1024], F32, tag="pqk", name=f"pq1_{h}_{i}")
                    for jj in range(w1 // 512):
                        nc.tensor.matmul(
                            pq1[:, ts(jj, 512)],
                            kT[hp, ts(i, 128)],
                            qT[hp, ts(j0 + jj, 512)],
                            start=True, stop=True,
                        )
                    # masked diag triangle -> bf16 SBUF, then write back into
                    # the psum row so the exp sweep sees masked values
                    dcol = 128 * r
                    nc.vector.tensor_tensor(
                        tri[h][i][:], pq1[:, dcol:dcol + 128], tm_sb[:], OP.mult
                    )
                    nc.vector.tensor_copy(pq1[:, dcol:dcol + 128], tri[h][i][:])
                    e1 = scratch.tile([128, 1024], BF16, tag="exp1",
                                      name=f"e1_{h}_{i}")
                    nc.scalar.activation(
                        e1[:, : w1 - dcol], pq1[:, dcol:w1], AF.Exp,
                        scale=0.125, accum_out=sums_tri[h][:, i:i + 1],
                    )
                    if width > 1024:
                        pq2 = pqk.tile([128, 1024], F32, tag="pqk",
                                       name=f"pq2_{h}_{i}")
                        for jj in range((width - 1024) // 512):
                            nc.tensor.matmul(
                                pq2[:, ts(jj, 512)],
                                kT[hp, ts(i, 128)],
                                qT[hp, ts(j0 + 2 + jj, 512)],
                                start=True, stop=True,
                            )
                        e2 = scratch.tile([128, 1024], BF16, tag="exp2",
                                          name=f"e2_{h}_{i}")
                        nc.scalar.activation(
                            e2[:, : width - 1024], pq2[:, : width - 1024],
                            AF.Exp, scale=0.125,
                            accum_out=sums_sfx[h][:, i:i + 1],
                        )

        # ---- Phase 1b: V projection + k/v transposes + prefix C/P ----
        nc.sync.dma_start(wbf2[:], wbf2_d[:])
        ps = [pp.tile([128, 512], F32, tag="pp", name=f"pp_v{j}")
              for j in range(4)]
        for o in range(8):
            chunk = instream.tile([128, S], BF16, tag="in")
            nc.sync.dma_start(chunk[:], vt_d[ts(o, 128), :])
            for j in range(4):
                nc.tensor.matmul(
                    ps[j][:], wv_sb[:, o, :], chunk[:, ts(j, 512)],
                    start=(o == 0), stop=(o == 7),
                )
        for j in range(4):
            nc.scalar.activation(
                vT[:, ts(j, 512)], ps[j][:], AF.Identity,
                bias=bv_sb[:], scale=1.0,
            )

    with tc.tile_pool(name="pt", bufs=2, space="PSUM") as pt, \
         tc.tile_pool(name="pc", bufs=2, space="PSUM") as pc:
        for srcT, dst in ((kT, k_sb), (vT, v_sb)):
            for m in range(16):
                ptile = pt.tile([128, 128], BF16, tag="pt", name=f"pt_{m}")
                nc.tensor.transpose(ptile[:], srcT[:, ts(m, 128)], ident[:])
                nc.vector.tensor_copy(dst[:, m, :], ptile[:])

        nc.vector.memset(p_f32[:, 0, :], 0.0)
        nc.vector.tensor_copy(p_bf[:, 0, :], p_f32[:, 0, :])
        for m in range(15):
            ctile = pc.tile([128, 64], F32, tag="pc", name=f"pc_{m}")
            nc.tensor.matmul(
                ctile[0:64, :], k_sb[:, m, 0:64], v_sb[:, m, 0:64],
                start=True, stop=True,
            )
            nc.tensor.matmul(
                ctile[64:128, :], k_sb[:, m, 64:128], v_sb[:, m, 64:128],
                start=True, stop=True, tile_position=(0, 64),
            )
            nc.vector.tensor_tensor(
                p_f32[:, m + 1, :], p_f32[:, m, :], ctile[:], OP.add
            )
            nc.vector.tensor_copy(p_bf[:, m + 1, :], p_f32[:, m + 1, :])

    # ---- Phase 3: lse, corr ----
    with tc.tile_pool(name="pcr", bufs=1, space="PSUM") as pcr:
        for h in range(2):
            nc.vector.tensor_tensor(
                lse_f[h][:], sums_tri[h][:], sums_sfx[h][:], OP.add
            )
            nc.vector.tensor_tensor(lse_f[h][:], lse_f[h][:], ct_sb[:], OP.add)
            nc.scalar.activation(lse_f[h][:], lse_f[h][:], AF.Ln, scale=1.0)
            nc.vector.tensor_copy(lse_b[h][:], lse_f[h][:])
        cps = pcr.tile([128, 1], F32, tag="pcr")
        for i in range(16):
            nc.tensor.matmul(
                cps[0:64, :], v_sb[:, i, 0:64], lse_b[0][:, i:i + 1],
                start=(i == 0), stop=(i == 15),
            )
            nc.tensor.matmul(
                cps[64:128, :], v_sb[:, i, 64:128], lse_b[1][:, i:i + 1],
                start=(i == 0), stop=(i == 15), tile_position=(0, 64),
            )
        nc.vector.tensor_copy(corr_sb[:], cps[:])

    # ---- Phase 4: attention assembly + WO, grouped 4 s-chunks per bank ----
    with tc.tile_pool(name="pat", bufs=1, space="PSUM") as pat, \
         tc.tile_pool(name="pwo", bufs=2, space="PSUM") as pwo:
        for g in range(4):
            patt = pat.tile([128, 512], F32, tag="pat", name=f"pat_{g}")
            for mm in range(4):
                m = 4 * g + mm
                cols = ts(mm, 128)
                if m > 0:
                    nc.tensor.matmul(
                        patt[0:64, cols], p_bf[0:64, m, :],
                        qT[0:64, ts(m, 128)], start=True, stop=False,
                    )
                    nc.tensor.matmul(
                        patt[64:128, cols], p_bf[64:128, m, :],
                        qT[64:128, ts(m, 128)],
                        start=True, stop=False, tile_position=(64, 64),
                    )
                nc.tensor.matmul(
                    patt[0:64, cols], v_sb[:, m, 0:64], tri[0][m][:],
                    start=(m == 0), stop=True,
                )
                nc.tensor.matmul(
                    patt[64:128, cols], v_sb[:, m, 64:128], tri[1][m][:],
                    start=(m == 0), stop=True, tile_position=(0, 64),
                )
            nc.vector.tensor_scalar(
                zT[:, ts(g, 512)], patt[:], 0.125, corr_sb[:],
                op0=OP.mult, op1=OP.subtract,
            )
            for mm in range(4):
                m = 4 * g + mm
                o_sb = outs.tile([128, D], BF16, tag="osb", name=f"osb_{m}")
                for half in range(2):
                    cols = slice(512 * half, 512 * half + 512)
                    po = pwo.tile([128, 512], F32, tag="pwo",
                                  name=f"pwo_{m}_{half}")
                    nc.tensor.matmul(
                        po[:], zT[:, ts(m, 128)], wo_sb[:, cols],
                        start=True, stop=True,
                    )
                    if half == 0:
                        nc.vector.tensor_copy(o_sb[:, cols], po[:])
                    else:
                        nc.scalar.copy(o_sb[:, cols], po[:])
                nc.sync.dma_start(out_d[ts(m, 128), :], o_sb[:])


def _get_program(loop_n=1):
    key = f"nc{loop_n}"
    if key not in _CACHE:
        _CACHE[key] = _build(loop_n)
    return _CACHE[key]


def _get_exec(loop_n=1):
    """Build the sharded PJRT executable once (same lowering path as
    concourse.bass2jax.run_bass_via_pjrt, hoisted so repeat calls don't
    re-trace/re-compile)."""
    key = f"exec{loop_n}"
    if key in _CACHE:
        return _CACHE[key]
    import jax
    import numpy as _np
    from jax.experimental.shard_map import shard_map
    from jax.sharding import Mesh, PartitionSpec
    import concourse.mybir as mybir
    from concourse import bass2jax

    nc = _get_program(loop_n)
    bass2jax.install_neuronx_cc_hook()

    partition_name = (
        nc.partition_id_tensor.name if nc.partition_id_tensor else None
    )
    in_names, out_names, out_avals = [], [], []
    for alloc in nc.m.functions[0].allocations:
        if not isinstance(alloc, mybir.MemoryLocationSet):
            continue
        name = alloc.memorylocations[0].name
        if alloc.kind == "ExternalInput":
            if name != partition_name:
                in_names.append(name)
        elif alloc.kind == "ExternalOutput":
            out_names.append(name)
            out_avals.append(
                jax.core.ShapedArray(
                    tuple(alloc.tensor_shape), mybir.dt.np(alloc.dtype)
                )
            )
    n_params = len(in_names)
    n_outs = len(out_avals)
    all_names = in_names + out_names
    if partition_name is not None:
        all_names = all_names + [partition_name]

    def _body(*args):
        operands = list(args)
        if partition_name is not None:
            operands.append(bass2jax.partition_id_tensor())
        outs = bass2jax._bass_exec_p.bind(
            *operands,
            out_avals=tuple(out_avals),
            in_names=tuple(all_names),
            out_names=tuple(out_names),
            lowering_input_output_aliases=(),
            sim_require_finite=True,
            sim_require_nnan=True,
            nc=nc,
        )
        return tuple(outs)

    devices = jax.devices()[:NCORES]
    mesh = Mesh(_np.asarray(devices), ("core",))
    donate = tuple(range(n_params, n_params + n_outs))
    sharded = jax.jit(
        shard_map(
            _body,
            mesh=mesh,
            in_specs=(PartitionSpec("core"),) * (n_params + n_outs),
            out_specs=(PartitionSpec("core"),) * n_outs,
            check_rep=False,
        ),
        donate_argnums=donate,
        keep_unused=True,
    )
    _CACHE[key] = (sharded, in_names, out_names, out_avals, mesh)
    return _CACHE[key]


def _run(in_maps, loop_n=1):
    """Execute on 8 cores; returns list of per-core output dicts."""
    import numpy as _np

    sharded, in_names, out_names, out_avals, mesh = _get_exec(loop_n)
    concat_in = [
        _np.concatenate([m[name] for m in in_maps], axis=0) for name in in_names
    ]
    concat_zeros = [
        _np.zeros((NCORES * a.shape[0], *a.shape[1:]), a.dtype) for a in out_avals
    ]
    out_arrs = sharded(*concat_in, *concat_zeros)
    return [
        {
            name: _np.asarray(out_arrs[i]).reshape(NCORES, *out_avals[i].shape)[c]
            for i, name in enumerate(out_names)
        }
        for c in range(NCORES)
    ]


def bench(in_maps, iters=5, loop_n=1):
    """Time device execution with device-resident inputs (excludes host
    transfer of the big operands; zero output buffers are pre-staged)."""
    import time

    import jax
    import numpy as _np
    from jax.sharding import NamedSharding, PartitionSpec

    sharded, in_names, out_names, out_avals, mesh = _get_exec(loop_n)
    sh = NamedSharding(mesh, PartitionSpec("core"))
    concat_in = [
        jax.device_put(
            _np.concatenate([m[name] for m in in_maps], axis=0), sh
        )
        for name in in_names
    ]
    zeros_pool = [
        [
            jax.device_put(
                _np.zeros((NCORES * a.shape[0], *a.shape[1:]), a.dtype), sh
            )
            for a in out_avals
        ]
        for _ in range(iters + 1)
    ]
    for z in zeros_pool:
        for a in z:
            a.block_until_ready()
    # warm-up
    outs = sharded(*concat_in, *zeros_pool[0])
    jax.block_until_ready(outs)
    times = []
    for it in range(iters):
        t0 = time.perf_counter()
        outs = sharded(*concat_in, *zeros_pool[it + 1])
        jax.block_until_ready(outs)
        times.append(time.perf_counter() - t0)
    return times, outs


def kernel(Q_input, K_input, V_input, WQw, WQb, WKw, WKb, WVw, WVb, WOw, WOb,
           _return_results=False):
    import ml_dtypes

    BF = ml_dtypes.bfloat16

    qt = np.ascontiguousarray(np.asarray(Q_input, np.float32).T).astype(BF)
    kt = np.ascontiguousarray(np.asarray(K_input, np.float32).T).astype(BF)
    vt = np.ascontiguousarray(np.asarray(V_input, np.float32).T).astype(BF)

    # triangular keep-mask M[p, c] = 1 if c >= p, and per-chunk skip counts
    tm = (np.arange(128)[None, :] >= np.arange(128)[:, None]).astype(np.float32)
    ct = np.broadcast_to(
        (128.0 * np.arange(16, dtype=np.float32))[None, :], (128, 16)
    ).copy()

    in_maps = []
    for c in range(NCORES):
        h0 = 2 * c
        def _prep_w(w):
            # [2, 64, D] -> [D, 128] -> partition-major [128, 8, 128]
            wt = np.asarray(w, np.float32).transpose(2, 0, 1).reshape(D, 128)
            return np.ascontiguousarray(
                wt.reshape(8, 128, 128).transpose(1, 0, 2)
            ).astype(BF)

        wq = _prep_w(WQw[h0:h0 + 2])
        wk = _prep_w(WKw[h0:h0 + 2])
        wv = _prep_w(WVw[h0:h0 + 2])
        wo = np.ascontiguousarray(
            np.asarray(WOw, np.float32)[:, 128 * c:128 * (c + 1)].T
        ).astype(BF)
        wbf1 = np.concatenate(
            [wq.reshape(128, 1024), wk.reshape(128, 1024)], axis=1
        )
        wbf2 = np.concatenate([wv.reshape(128, 1024), wo], axis=1)
        cf = np.concatenate(
            [
                np.asarray(WQb[h0:h0 + 2], np.float32).reshape(128, 1),
                np.asarray(WKb[h0:h0 + 2], np.float32).reshape(128, 1),
                np.asarray(WVb[h0:h0 + 2], np.float32).reshape(128, 1),
                tm, ct,
            ],
            axis=1,
        )
        in_maps.append({
            "qt": qt, "kt": kt, "vt": vt,
            "wbf1": wbf1, "wbf2": wbf2, "cf": np.ascontiguousarray(cf),
        })

    results = _run(in_maps)
    out = np.zeros((S, D), np.float64)
    for c in range(NCORES):
        out += results[c]["out"].astype(np.float64)
    out += np.asarray(WOb, np.float32)[None, :]
    if _return_results:
        return out.astype(np.float32), (results, in_maps)
    return out.astype(np.float32)



# revision 43
# speedup vs baseline: 1.1053x; 1.1053x over previous
"""Trainium2 Bass kernel for the 16-head masked-attention module (v2).

Math per head (reference):
    q = Q @ Wq.T + bq ; k = K @ Wk.T + bk ; v = V @ Wv.T + bv      [S, 64]
    qk = tril(q @ k.T)                 (zeroed, not -inf)
    scores = log_softmax(qk / 8, axis=0)   (softmax over the QUERY axis,
                                            per key column)
    attn = scores @ v
    out = concat(heads) @ WO.T + bO

Decomposition (8 cores, 2 heads/core, tensor-parallel over heads, WO
row-sharded):

    scores[s,t] = msc[t,s]/8 - lse[t],  lse[t] = log(t + sum_{s>=t} exp(msc/8))
    attn[s,:]   = (masked qk/8) @ v  -  ones * (lse @ v)

    The lse part is RANK-1 in the output (same vector for every row s), so the
    device ships out_nocorr = ((masked qk/8)@v)@WOc^T in f32 (DMA'd straight
    from PSUM) plus corrout = (lse@v)@WOc^T [1,1024]; the host subtracts the
    broadcast correction and adds bO.  This decouples the whole exp/lse sweep
    from the output path - it runs concurrently on the Scalar engine.

    Projections run in fp8e4m3 with DoubleRow matmuls (inputs and weights
    quantized on host; f32 accumulate).  K is pre-scaled by 1/8 in its
    projection epilogue so score psum is msc/8 directly.  Fully-kept 128-blocks
    of the score triangle collapse via rank-64 prefix sums (C_m = k_m^T v_m);
    only diagonal blocks are materialized (masked into SBUF, rewritten into
    PSUM by an identity matmul so the exp sweep sees masked values).
"""

import numpy as np

S = 2048
D = 1024
NCORES = 8

_CACHE = {}


def _split_multi_waits(nc, mybir, max_waits=1):
    """This walrus build only encodes one sync-wait per instruction; Tile's
    tail drain carries several. Hoist extras onto preceding NoOps."""
    n = 0
    for fn in nc.m.functions:
        for blk in fn.blocks:
            out = []
            changed = False
            for ins in blk.instructions:
                si = getattr(ins, "sync_info", None)
                waits = list(si.on_wait) if (si is not None and si.on_wait) else []
                if len(waits) > max_waits:
                    for w in waits[:-max_waits]:
                        nop = mybir.InstNoOp(
                            name=nc.get_next_instruction_name(), ins=[], outs=[]
                        )
                        nop.engine = ins.engine
                        nop.sync_info = mybir.SyncInfo(on_wait=[w], on_update=[])
                        out.append(nop)
                        n += 1
                    si.on_wait = waits[-max_waits:]
                    changed = True
                out.append(ins)
            if changed:
                blk.instructions = out
    return n


def _build(loop_n=1):
    import concourse.bass as bass
    import concourse.mybir as mybir
    import concourse.tile as tile
    from concourse.bass import ts
    from concourse.masks import make_identity

    F32 = mybir.dt.float32
    BF16 = mybir.dt.bfloat16
    FP8 = mybir.dt.float8e4
    AF = mybir.ActivationFunctionType
    OP = mybir.AluOpType
    DR = mybir.MatmulPerfMode.DoubleRow

    nc = bass.Bass("TRN2", num_devices=NCORES, debug=False)

    qf_d = nc.dram_tensor("qf8", [D, S], FP8, kind="ExternalInput")
    kf_d = nc.dram_tensor("kf8", [D, S], FP8, kind="ExternalInput")
    # V stays bf16: attn multiplies v by the large lse weights, so fp8 V
    # noise lands directly in the output (rel err 2e-2); Q/K noise washes
    # out through the softmax (4e-3)
    vf_d = nc.dram_tensor("vtb", [D, S], BF16, kind="ExternalInput")
    # packed DoubleRow weights [wq|wk]: free index = o*256 + jj*128 + m,
    # value = W[m, 256*o + 128*jj + p]
    wf8_d = nc.dram_tensor("wf8", [128, 2048], FP8, kind="ExternalInput")
    # [wv(bf16, o-chunk packed) | wo]
    wo_d = nc.dram_tensor("wob", [128, 2048], BF16, kind="ExternalInput")
    # cf = [bq | bk/8 | bv | tm(128) | ct(16)] (f32)
    cf_d = nc.dram_tensor("cf", [128, 147], F32, kind="ExternalInput")
    out_d = nc.dram_tensor("out", [S, D], BF16, kind="ExternalOutput")
    corr_d = nc.dram_tensor("corrout", [1, D], F32, kind="ExternalOutput")

    with tile.TileContext(nc) as tc:
        with (
            tc.tile_pool(name="singles", bufs=1) as sg,
            tc.tile_pool(name="stage", bufs=5) as stage,
            tc.tile_pool(name="vstage", bufs=4) as vstage,
            tc.tile_pool(name="outs", bufs=3) as outs,
        ):
            wf8 = sg.tile([128, 2048], FP8, tag="wf8")
            wob = sg.tile([128, 2048], BF16, tag="wob")
            cf = sg.tile([128, 147], F32, tag="cf")
            ident = sg.tile([128, 128], BF16, tag="ident")
            thoist = sg.tile([128, 1], F32, tag="thoist")
            nc.sync.dma_start(cf[:], cf_d[:])
            nc.sync.dma_start(wf8[:], wf8_d[:])
            make_identity(nc, ident[:])
            # dummy exp: pulls the ~1.3us activation-table load off the
            # first real score row
            nc.scalar.activation(thoist[:], ident[:, 0:1], AF.Exp, scale=1.0)

            wq_v = wf8[:, 0:1024].rearrange("p (o jj m) -> p o jj m", o=4, jj=2)
            wk_v = wf8[:, 1024:2048].rearrange("p (o jj m) -> p o jj m", o=4, jj=2)
            wv_v = wob[:, 0:1024].rearrange("p (o m) -> p o m", o=8)
            wo_sb = wob[:, 1024:2048]
            bq = cf[:, 0:1]
            bk8 = cf[:, 1:2]
            bv = cf[:, 2:3]
            tm_sb = cf[:, 3:131]
            ct_sb = cf[:, 131:147]

            qT = sg.tile([128, S], BF16, tag="qT")   # [dk(2 heads), s]
            kT = sg.tile([128, S], BF16, tag="kT")   # pre-scaled by 1/8
            vT = sg.tile([128, S], BF16, tag="vT")
            k_sb = sg.tile([128, 16, 128], BF16, tag="k_sb")  # [t, chunk, dk]
            v_sb = sg.tile([128, 16, 128], BF16, tag="v_sb")
            p_f32 = sg.tile([128, 16, 64], F32, tag="p_f32")
            p_bf = sg.tile([128, 16, 64], BF16, tag="p_bf")
            zT = sg.tile([128, S], BF16, tag="zT")
            sums = [sg.tile([128, 16], F32, tag=f"sums{h}", name=f"sums{h}")
                    for h in range(2)]
            lse_b = [sg.tile([128, 16], BF16, tag=f"lse_b{h}", name=f"lse_b{h}")
                     for h in range(2)]
            corr_b = sg.tile([128, 1], BF16, tag="corr_b")
            corrv = sg.tile([1, 1024], F32, tag="corrv")
            tri = [
                [sg.tile([128, 128], BF16, tag=f"tri{h}_{i}", name=f"tri{h}_{i}")
                 for i in range(16)]
                for h in range(2)
            ]

            def emit_body():
                _emit_phases(
                    nc, tc, tile, mybir, ts, F32, BF16, FP8, AF, OP, DR,
                    qf_d, kf_d, vf_d, out_d, corr_d, wo_d, wob,
                    wq_v, wk_v, wv_v, wo_sb, bq, bk8, bv, tm_sb, ct_sb, ident,
                    stage, vstage, outs,
                    qT, kT, vT, k_sb, v_sb, p_f32, p_bf, zT,
                    sums, lse_b, corr_b, corrv, tri,
                )

            if loop_n == 1:
                emit_body()
            else:
                with tc.For_i(0, loop_n, 1):
                    emit_body()

    _split_multi_waits(nc, mybir)
    return nc


def _emit_phases(
    nc, tc, tile, mybir, ts, F32, BF16, FP8, AF, OP, DR,
    qf_d, kf_d, vf_d, out_d, corr_d, wo_d, wob,
    wq_v, wk_v, wv_v, wo_sb, bq, bk8, bv, tm_sb, ct_sb, ident,
    stage, vstage, outs,
    qT, kT, vT, k_sb, v_sb, p_f32, p_bf, zT,
    sums, lse_b, corr_b, corrv, tri,
):
    def in_slice(src_d, sr):
        # [128, o=4, jj=2, 512] fp8 slice: element (p,o,jj,s) =
        # src[256*o + 128*jj + p, 512*sr + s]
        v = src_d.rearrange("(o jj p) s -> p o jj s", o=4, jj=2)
        return v[:, :, :, ts(sr, 512)]

    def proj_slice(psum, w_v, st_v, sr):
        for o in range(4):
            nc.tensor.matmul(
                psum[:], w_v[:, o], st_v[:, o],
                start=(o == 0), stop=(o == 3), perf_mode=DR,
            )

    def proj_slice_v(psum, st_v):
        for o in range(8):
            nc.tensor.matmul(
                psum[:], wv_v[:, o], st_v[:, o],
                start=(o == 0), stop=(o == 7),
            )

    # Helpers shared by both psum scopes.  Rows live in rotating psum tiles
    # whose column c maps to score column base+c; groups carve attention,
    # WO, and prefix-C psum out of the same rotation.
    vst = []

    def emit_vdma():
        # V slices are bf16: [128, o=8, 512] per slice
        vv = vf_d.rearrange("(o p) s -> p o s", o=8)
        for sr in range(4):
            st = vstage.tile([128, 4096], BF16, tag="vin", name=f"st_v{sr}")
            vst.append(st.rearrange("p (o s) -> p o s", o=8))
            nc.sync.dma_start(vst[sr][:], vv[:, :, ts(sr, 512)])
        nc.sync.dma_start(wob[:], wo_d[:])

    def emit_row(pool, width, base, i, h):
        j0 = i // 4
        c0 = 128 * i
        hp = slice(64 * h, 64 * h + 64)
        t = pool.tile([128, width], F32, tag="qk", name=f"qk_{h}_{i}")
        w0 = 512 * (j0 + 1) - c0
        nc.tensor.matmul(
            t[:, c0 - base:c0 - base + w0], kT[hp, ts(i, 128)],
            qT[hp, c0:c0 + w0], start=True, stop=True,
        )
        for j in range(j0 + 1, 4):
            nc.tensor.matmul(
                t[:, 512 * j - base:512 * j - base + 512],
                kT[hp, ts(i, 128)], qT[hp, ts(j, 512)],
                start=True, stop=True,
            )
        nc.vector.tensor_tensor(
            tri[h][i][:], t[:, c0 - base:c0 - base + 128], tm_sb, OP.mult
        )
        nc.tensor.matmul(
            t[:, c0 - base:c0 - base + 128], ident[:], tri[h][i][:],
            start=True, stop=True,
        )
        nc.scalar.activation(
            t[:, c0 - base:], t[:, c0 - base:], AF.Exp,
            scale=1.0, accum_out=sums[h][:, i:i + 1],
        )

    def emit_chain(ct1, ct2):
        # ALL C_m matmuls back-to-back (no cross-engine serialization on the
        # PE queue), then the serial P-prefix chain on DVE + Pool alone.
        nc.vector.memset(p_f32[:, 0, :], 0.0)
        nc.gpsimd.tensor_copy(p_bf[:, 0, :], p_f32[:, 0, :])

        def slot(m):
            return (ct1 if m < 8 else ct2)[:, 64 * (m % 8):64 * (m % 8) + 64]

        for m in range(15):
            ct = slot(m)
            nc.tensor.matmul(
                ct[0:64, :], k_sb[:, m, 0:64], v_sb[:, m, 0:64],
                start=True, stop=True,
            )
            nc.tensor.matmul(
                ct[64:128, :], k_sb[:, m, 64:128], v_sb[:, m, 64:128],
                start=True, stop=True, tile_position=(0, 64),
            )
        for m in range(15):
            nc.vector.tensor_tensor(
                p_f32[:, m + 1, :], p_f32[:, m, :], slot(m), OP.add
            )
            nc.gpsimd.tensor_copy(p_bf[:, m + 1, :], p_f32[:, m + 1, :])

    def emit_attn(g, attn_t):
        # attention assembly into attn_t[:, 0:512], then zT slice
        for mm in range(4):
            m = 4 * g + mm
            cols = slice(128 * mm, 128 * mm + 128)
            if m > 0:
                nc.tensor.matmul(
                    attn_t[0:64, cols], p_bf[0:64, m, :], qT[0:64, ts(m, 128)],
                    start=True, stop=False,
                )
                nc.tensor.matmul(
                    attn_t[64:128, cols], p_bf[64:128, m, :],
                    qT[64:128, ts(m, 128)],
                    start=True, stop=False, tile_position=(64, 64),
                )
            nc.tensor.matmul(
                attn_t[0:64, cols], v_sb[:, m, 0:64], tri[0][m][:],
                start=(m == 0), stop=True,
            )
            nc.tensor.matmul(
                attn_t[64:128, cols], v_sb[:, m, 64:128], tri[1][m][:],
                start=(m == 0), stop=True, tile_position=(0, 64),
            )
        nc.vector.tensor_copy(zT[:, ts(g, 512)], attn_t[:, 0:512])

    def emit_wo(g, mm, wo_tiles, act_halves=(False, False), wide=None):
        # WO for one 128-row output chunk via two [128,512] psum halves;
        # bf16 staging copies (one wide copy if `wide` tile given), DMA out
        m = 4 * g + mm
        o_sb = outs.tile([128, 1024], BF16, tag="osb", name=f"osb_{m}")
        for half in range(2):
            po = wo_tiles[half]
            nc.tensor.matmul(
                po[:], zT[:, ts(m, 128)], wo_sb[:, ts(half, 512)],
                start=True, stop=True,
            )
            if wide is None:
                if act_halves[half]:
                    nc.scalar.copy(o_sb[:, ts(half, 512)], po[:])
                else:
                    nc.vector.tensor_copy(o_sb[:, ts(half, 512)], po[:])
        if wide is not None:
            if act_halves[0]:
                nc.scalar.copy(o_sb[:], wide[:])
            else:
                nc.vector.tensor_copy(o_sb[:], wide[:])
        nc.sync.dma_start(out_d[ts(m, 128), :], o_sb[:])

    def emit_row_pair(pool, i):
        # rows 12-15: both heads packed in one [128,1024] tile (512-wide
        # halves) so the rotation distance doubles for the short exps
        c0 = 128 * i
        t = pool.tile([128, 1024], F32, tag="qk", name=f"qkp_{i}")
        for h in range(2):
            hp = slice(64 * h, 64 * h + 64)
            lo = 512 * h + c0 - 1536
            nc.tensor.matmul(
                t[:, lo:512 * h + 512], kT[hp, ts(i, 128)], qT[hp, c0:2048],
                start=True, stop=True,
            )
            nc.vector.tensor_tensor(
                tri[h][i][:], t[:, lo:lo + 128], tm_sb, OP.mult
            )
            nc.tensor.matmul(
                t[:, lo:lo + 128], ident[:], tri[h][i][:],
                start=True, stop=True,
            )
            nc.scalar.activation(
                t[:, lo:512 * h + 512], t[:, lo:512 * h + 512], AF.Exp,
                scale=1.0, accum_out=sums[h][:, i:i + 1],
            )

    # ---- phase A1: [128,2048] x 2 (8 banks): Q/K projections + rows 0-3.
    # Q is DMA'd first (every score row needs all of qT); K follows with
    # per-slice epilogues interleaved between the first rows so row 0
    # unblocks right after K slice 0 lands; V DMAs are queued last. ----
    with tc.tile_pool(name="pqkA", bufs=2, space="PSUM") as pqkA:
        qst = []
        for sr in range(4):
            st = stage.tile([128, 4096], FP8, tag="in", name=f"st_q{sr}")
            qst.append(st.rearrange("p (o jj s) -> p o jj s", o=4, jj=2))
            nc.sync.dma_start(qst[sr][:], in_slice(qf_d, sr))
        qp = pqkA.tile([128, 2048], F32, tag="qk", name="pp_q")
        for sr in range(4):
            proj_slice(qp[:, ts(sr, 512)], wq_v, qst[sr], sr)
            nc.vector.tensor_scalar(
                qT[:, ts(sr, 512)], qp[:, ts(sr, 512)], bq, None, op0=OP.add,
            )

        kp = pqkA.tile([128, 2048], F32, tag="qk", name="pp_k")

        def emit_kslice(sr):
            st = stage.tile([128, 4096], FP8, tag="in", name=f"st_k{sr}")
            st_v = st.rearrange("p (o jj s) -> p o jj s", o=4, jj=2)
            nc.sync.dma_start(st_v[:], in_slice(kf_d, sr))
            proj_slice(kp[:, ts(sr, 512)], wk_v, st_v, sr)
            nc.vector.tensor_scalar(
                kT[:, ts(sr, 512)], kp[:, ts(sr, 512)], bk8, 0.125,
                op0=OP.add, op1=OP.mult,
            )

        emit_kslice(0)
        emit_row(pqkA, 2048, 0, 0, 0)
        emit_kslice(1)
        emit_row(pqkA, 2048, 0, 0, 1)
        emit_kslice(2)
        emit_row(pqkA, 2048, 0, 1, 0)
        emit_kslice(3)
        emit_vdma()
        # k_sb transposes depend only on kT (XBAR DMA, SBUF->SBUF)
        for m in range(16):
            nc.sync.dma_start_transpose(k_sb[:, m, :], kT[:, ts(m, 128)])
        emit_row(pqkA, 2048, 0, 1, 1)
        for i in range(2, 4):
            emit_row(pqkA, 2048, 0, i, 0)
            emit_row(pqkA, 2048, 0, i, 1)

    # ---- phases A2/A3 under gpool: gpool ([128,512] x 2, banks 6-7) feeds
    # the group stream (C slices, attention, WO halves) independently of
    # the Act-paced row rotation, so outputs flow from ~t+25us on.  A2
    # [128,1536] x 2 runs V + rows 4-7; A3 [128,1024] x 3 runs rows 8-15
    # (12-15 head-packed) at effective depth 3+. ----
    with tc.tile_pool(name="gpool", bufs=2, space="PSUM") as gpool:

        def gp(name):
            return gpool.tile([128, 512], F32, tag="gp", name=name)

        def attnG(g):
            emit_attn(g, gp(f"grp{g}a"))

        def woG(g, mm, act_halves=(False, False)):
            emit_wo(g, mm, [gp(f"g{g}w{mm}h0"), gp(f"g{g}w{mm}h1")],
                    act_halves)

        with tc.tile_pool(name="pqkA2", bufs=2, space="PSUM") as pqkA2:
            # rows 4-5 first (their tiles alias A1's last rows), then V
            # projection in ONE [128,1536] tile: slices 0-2 in place,
            # slice 3 reuses [0:512] after its epilogue; epilogues and
            # v_sb transposes interleave between row emissions so no
            # single DVE op blocks the row masks for long
            emit_row(pqkA2, 1536, 512, 4, 0)
            emit_row(pqkA2, 1536, 512, 4, 1)
            vt1 = pqkA2.tile([128, 1536], F32, tag="qk", name="vproj")

            def vepi(sr):
                src = vt1[:, ts(sr, 512)] if sr < 3 else vt1[:, 0:512]
                nc.vector.tensor_scalar(
                    vT[:, ts(sr, 512)], src, bv, None, op0=OP.add,
                )
                for m in range(4 * sr, 4 * sr + 4):
                    nc.sync.dma_start_transpose(
                        v_sb[:, m, :], vT[:, ts(m, 128)]
                    )

            for sr in range(3):
                proj_slice_v(vt1[:, ts(sr, 512)], vst[sr])
            vepi(0)
            emit_row(pqkA2, 1536, 512, 5, 0)
            vepi(1)
            emit_row(pqkA2, 1536, 512, 5, 1)
            vepi(2)
            proj_slice_v(vt1[:, 0:512], vst[3])
            vepi(3)
            emit_chain(gp("ct1"), gp("ct2"))
            emit_row(pqkA2, 1536, 512, 6, 0)
            attnG(0)
            emit_row(pqkA2, 1536, 512, 6, 1)
            woG(0, 0)
            emit_row(pqkA2, 1536, 512, 7, 0)
            woG(0, 1)
            emit_row(pqkA2, 1536, 512, 7, 1)
            woG(0, 2)

        with tc.tile_pool(name="pqkB", bufs=3, space="PSUM") as pqkB:

            def woB(g, mm, on_act=False):
                gw = pqkB.tile([128, 1024], F32, tag="qk",
                               name=f"g{g}w{mm}")
                emit_wo(g, mm, [gw[:, 0:512], gw[:, 512:1024]],
                        act_halves=(on_act, on_act), wide=gw)

            woG(0, 3)
            emit_row(pqkB, 1024, 1024, 8, 0)
            emit_row(pqkB, 1024, 1024, 8, 1)
            attnG(1)
            emit_row(pqkB, 1024, 1024, 9, 0)
            woB(1, 0)
            emit_row(pqkB, 1024, 1024, 9, 1)
            woB(1, 1)
            emit_row(pqkB, 1024, 1024, 10, 0)
            woB(1, 2)
            emit_row(pqkB, 1024, 1024, 10, 1)
            woB(1, 3)
            emit_row(pqkB, 1024, 1024, 11, 0)
            emit_row(pqkB, 1024, 1024, 11, 1)
            attnG(2)
            emit_row_pair(pqkB, 12)
            woB(2, 0)
            emit_row_pair(pqkB, 13)
            woB(2, 1)
            emit_row_pair(pqkB, 14)
            woB(2, 2, on_act=True)
            emit_row_pair(pqkB, 15)
            woB(2, 3)
            attnG(3)
            woB(3, 0, on_act=True)
            woB(3, 1)
            woB(3, 2, on_act=True)
            woB(3, 3)

    # ---- lse / corr tail ----
    with tc.tile_pool(name="pcr", bufs=1, space="PSUM") as pcr:
        for h in range(2):
            nc.vector.tensor_tensor(sums[h][:], sums[h][:], ct_sb, OP.add)
            nc.scalar.activation(sums[h][:], sums[h][:], AF.Ln, scale=1.0)
            nc.vector.tensor_copy(lse_b[h][:], sums[h][:])
        cps = pcr.tile([128, 1024], F32, tag="pcr")
        for i in range(16):
            nc.tensor.matmul(
                cps[0:64, 0:1], v_sb[:, i, 0:64], lse_b[0][:, i:i + 1],
                start=(i == 0), stop=(i == 15),
            )
            nc.tensor.matmul(
                cps[64:128, 0:1], v_sb[:, i, 64:128], lse_b[1][:, i:i + 1],
                start=(i == 0), stop=(i == 15), tile_position=(0, 64),
            )
        nc.vector.tensor_copy(corr_b[:], cps[:, 0:1])
        o_c = outs.tile([1, 1024], F32, tag="corr_ps", name="corr_fin")
        for half in range(2):
            nc.tensor.matmul(
                cps[0:1, ts(half, 512)], corr_b[:], wo_sb[:, ts(half, 512)],
                start=True, stop=True,
            )
        nc.vector.tensor_copy(o_c[:], cps[0:1, :])
        nc.sync.dma_start(corr_d[:], o_c[:])


def _get_program(loop_n=1):
    key = f"nc{loop_n}"
    if key not in _CACHE:
        _CACHE[key] = _build(loop_n)
    return _CACHE[key]


def _get_exec(loop_n=1):
    """Build the sharded PJRT executable once (same lowering path as
    concourse.bass2jax.run_bass_via_pjrt, hoisted so repeat calls don't
    re-trace/re-compile)."""
    key = f"exec{loop_n}"
    if key in _CACHE:
        return _CACHE[key]
    import jax
    import numpy as _np
    from jax.experimental.shard_map import shard_map
    from jax.sharding import Mesh, PartitionSpec
    import concourse.mybir as mybir
    from concourse import bass2jax

    nc = _get_program(loop_n)
    bass2jax.install_neuronx_cc_hook()

    partition_name = (
        nc.partition_id_tensor.name if nc.partition_id_tensor else None
    )
    in_names, out_names, out_avals = [], [], []
    for alloc in nc.m.functions[0].allocations:
        if not isinstance(alloc, mybir.MemoryLocationSet):
            continue
        name = alloc.memorylocations[0].name
        if alloc.kind == "ExternalInput":
            if name != partition_name:
                in_names.append(name)
        elif alloc.kind == "ExternalOutput":
            out_names.append(name)
            out_avals.append(
                jax.core.ShapedArray(
                    tuple(alloc.tensor_shape), mybir.dt.np(alloc.dtype)
                )
            )
    n_params = len(in_names)
    n_outs = len(out_avals)
    all_names = in_names + out_names
    if partition_name is not None:
        all_names = all_names + [partition_name]

    def _body(*args):
        operands = list(args)
        if partition_name is not None:
            operands.append(bass2jax.partition_id_tensor())
        outs = bass2jax._bass_exec_p.bind(
            *operands,
            out_avals=tuple(out_avals),
            in_names=tuple(all_names),
            out_names=tuple(out_names),
            lowering_input_output_aliases=(),
            sim_require_finite=True,
            sim_require_nnan=True,
            nc=nc,
        )
        return tuple(outs)

    devices = jax.devices()[:NCORES]
    mesh = Mesh(_np.asarray(devices), ("core",))
    donate = tuple(range(n_params, n_params + n_outs))
    sharded = jax.jit(
        shard_map(
            _body,
            mesh=mesh,
            in_specs=(PartitionSpec("core"),) * (n_params + n_outs),
            out_specs=(PartitionSpec("core"),) * n_outs,
            check_rep=False,
        ),
        donate_argnums=donate,
        keep_unused=True,
    )
    _CACHE[key] = (sharded, in_names, out_names, out_avals, mesh)
    return _CACHE[key]


def _run(in_maps, loop_n=1):
    """Execute on 8 cores; returns list of per-core output dicts."""
    import numpy as _np

    sharded, in_names, out_names, out_avals, mesh = _get_exec(loop_n)
    concat_in = [
        _np.concatenate([m[name] for m in in_maps], axis=0) for name in in_names
    ]
    concat_zeros = [
        _np.zeros((NCORES * a.shape[0], *a.shape[1:]), a.dtype) for a in out_avals
    ]
    out_arrs = sharded(*concat_in, *concat_zeros)
    return [
        {
            name: _np.asarray(out_arrs[i]).reshape(NCORES, *out_avals[i].shape)[c]
            for i, name in enumerate(out_names)
        }
        for c in range(NCORES)
    ]


def bench(in_maps, iters=5, loop_n=1):
    """Time device execution with device-resident inputs (excludes host
    transfer of the big operands; zero output buffers are pre-staged)."""
    import time

    import jax
    import numpy as _np
    from jax.sharding import NamedSharding, PartitionSpec

    sharded, in_names, out_names, out_avals, mesh = _get_exec(loop_n)
    sh = NamedSharding(mesh, PartitionSpec("core"))
    concat_in = [
        jax.device_put(
            _np.concatenate([m[name] for m in in_maps], axis=0), sh
        )
        for name in in_names
    ]
    zeros_pool = [
        [
            jax.device_put(
                _np.zeros((NCORES * a.shape[0], *a.shape[1:]), a.dtype), sh
            )
            for a in out_avals
        ]
        for _ in range(iters + 1)
    ]
    for z in zeros_pool:
        for a in z:
            a.block_until_ready()
    # warm-up
    outs = sharded(*concat_in, *zeros_pool[0])
    jax.block_until_ready(outs)
    times = []
    for it in range(iters):
        t0 = time.perf_counter()
        outs = sharded(*concat_in, *zeros_pool[it + 1])
        jax.block_until_ready(outs)
        times.append(time.perf_counter() - t0)
    return times, outs


def kernel(Q_input, K_input, V_input, WQw, WQb, WKw, WKb, WVw, WVb, WOw, WOb,
           _return_results=False):
    import ml_dtypes

    BF = ml_dtypes.bfloat16
    F8 = ml_dtypes.float8_e4m3

    qf = np.ascontiguousarray(np.asarray(Q_input, np.float32).T).astype(F8)
    kf = np.ascontiguousarray(np.asarray(K_input, np.float32).T).astype(F8)
    vt = np.ascontiguousarray(np.asarray(V_input, np.float32).T).astype(BF)

    # triangular keep-mask M[p, c] = 1 if c >= p, and per-chunk skip counts
    tm = (np.arange(128)[None, :] >= np.arange(128)[:, None]).astype(np.float32)
    ct = np.broadcast_to(
        (128.0 * np.arange(16, dtype=np.float32))[None, :], (128, 16)
    ).copy()

    in_maps = []
    for c in range(NCORES):
        h0 = 2 * c

        def _prep_w8(w):
            # [2, 64, D] -> Wm [128 out, 1024 D] -> DoubleRow pack
            # [p, o, jj, m]: (p,o,jj,m) = Wm[m, 256*o + 128*jj + p]
            wm = np.asarray(w, np.float32).reshape(128, D)
            return np.ascontiguousarray(
                wm.reshape(128, 4, 2, 128).transpose(3, 1, 2, 0)
            ).astype(F8).reshape(128, 1024)

        def _prep_wb(w):
            # bf16 o-chunk pack: [p, o, m] = Wm[m, 128*o + p]
            wm = np.asarray(w, np.float32).reshape(128, D)
            return np.ascontiguousarray(
                wm.reshape(128, 8, 128).transpose(2, 1, 0)
            ).astype(BF).reshape(128, 1024)

        wq = _prep_w8(WQw[h0:h0 + 2])
        wk = _prep_w8(WKw[h0:h0 + 2])
        wf8 = np.ascontiguousarray(np.concatenate([wq, wk], axis=1))
        wo = np.asarray(WOw, np.float32)[:, 128 * c:128 * (c + 1)].T
        wob = np.ascontiguousarray(np.concatenate(
            [_prep_wb(WVw[h0:h0 + 2]).astype(np.float32), wo], axis=1
        )).astype(BF)
        cf = np.concatenate(
            [
                np.asarray(WQb[h0:h0 + 2], np.float32).reshape(128, 1),
                np.asarray(WKb[h0:h0 + 2], np.float32).reshape(128, 1) / 8.0,
                np.asarray(WVb[h0:h0 + 2], np.float32).reshape(128, 1),
                tm, ct,
            ],
            axis=1,
        )
        in_maps.append({
            "qf8": qf, "kf8": kf, "vtb": vt,
            "wf8": wf8, "wob": wob, "cf": np.ascontiguousarray(cf),
        })

    results = _run(in_maps)
    out = np.zeros((S, D), np.float64)
    corr = np.zeros((D,), np.float64)
    for c in range(NCORES):
        out += results[c]["out"].astype(np.float64)
        corr += results[c]["corrout"][0].astype(np.float64)
    out -= corr[None, :]
    out += np.asarray(WOb, np.float32)[None, :]
    if _return_results:
        return out.astype(np.float32), (results, in_maps)
    return out.astype(np.float32)
